# revision 1
# baseline (speedup 1.0000x reference)
# Trainium2 Bass kernel for nn_Attention_48052094107920 (sparse_attention).
#
# Math (see reference):
#   q,k: GH=3 global heads of dim 64; v: LH=12 local heads of dim 64
#   S_g = (x Wq)_g (x Wk)_g^T * scale                  [B,3,N,N]
#   mw  = (masks @ mask_proj).reshape(N,N,3,12)
#   A_h = sum_g S_g * mw[:,:,g,h]                      [B,12,N,N]
#   out = softmax_k(A_h) @ v_h  -> output projection + bias
#
# Default sharding (kernel_v2): core c = (head-group c//2, query-half c%2).
# Each core processes ALL 8 batches for its 3 heads and 320-column q-half and
# emits partial projection outputs; the host sums the 4 head-group partials
# and adds proj_b. This amortizes the batch-independent mask-mix maps
# mw[g,h] (the dominant Vector-engine work) over all 8 batches.
# kernel_v1 (kept below) is plain batch-parallel: 1 batch per core.
#
# Shared device-side design choices:
#   - "k-major" score layout: score tiles are S^T[k, q] (k on partitions) so
#     p @ v needs no transposes and the output projection receives its lhsT
#     (= o^T) directly from PSUM.
#   - x is host-transposed and augmented with a ones-row; W_v is augmented so
#     the v projection yields interleaved [v_h | ones] columns: the ones
#     column produces the softmax denominator Z during the p@v matmul.
#   - softmax skips max-subtraction (logits are O(5) here) and folds 1/Z in
#     after p@v; padded k-rows are killed with a -30 exp bias.
#   - fp16 everywhere on-chip (fp32 PSUM accumulation and fp32 Z / 1/Z);
#     mask_proj enters as per-partition scalar tiles, so no runtime values
#     are baked into the compiled program.

import numpy as np
import ml_dtypes

import concourse.bass as bass
import concourse.bacc as bacc_mod
import concourse.mybir as mybir
import concourse.tile as tile
from concourse import bass_utils

BF = mybir.dt.float16  # fp16: 10-bit mantissa, same engine speed as bf16
F32 = mybir.dt.float32
AF = mybir.ActivationFunctionType
OP = mybir.AluOpType

B, N, C = 8, 577, 768
GH, LH, ML, HD = 3, 12, 3, 64
SCALE = HD ** -0.5
NP = 640          # padded token count (5 * 128)
NJ = 5            # k sub-chunks of 128
CP = 896          # padded channel count (7 * 128)
KO = 7            # contraction sub-chunks for the qkv projections
QC = 128          # q-chunk width
NQC = NP // QC    # 5 q-chunks
VW = HD + 1       # 65: v head columns + ones column
EXP_NEG = -30.0   # exp bias for padded k rows


def build_nc(linearize=False):
    nc = bacc_mod.Bacc("TRN2", target_bir_lowering=False, debug=False, num_devices=8)

    xt = nc.dram_tensor("xt", [128, KO, NP], BF, kind="ExternalInput")
    wq = nc.dram_tensor("wq", [128, KO, GH * HD], BF, kind="ExternalInput")
    wk = nc.dram_tensor("wk", [128, KO, GH * HD], BF, kind="ExternalInput")
    wv = nc.dram_tensor("wv", [128, KO, LH * VW], BF, kind="ExternalInput")
    pw = nc.dram_tensor("pw", [64, LH, C], BF, kind="ExternalInput")
    pb = nc.dram_tensor("pb", [128, C], F32, kind="ExternalInput")
    mt = nc.dram_tensor("mt", [128, ML * NJ, NP], BF, kind="ExternalInput")
    cw = nc.dram_tensor("cw", [128, GH * ML * LH], F32, kind="ExternalInput")
    eb = nc.dram_tensor("eb", [128, 1], F32, kind="ExternalInput")
    out = nc.dram_tensor("o", [NP, C], F32, kind="ExternalOutput")

    with tile.TileContext(nc, linearize=linearize) as tc, \
         tc.tile_pool(name="const", bufs=1) as cpool, \
         tc.tile_pool(name="work", bufs=2) as wpool, \
         tc.tile_pool(name="attn", bufs=3) as apool, \
         tc.tile_pool(name="psA", bufs=2, space="PSUM") as ppA, \
         tc.tile_pool(name="psS", bufs=4, space="PSUM") as ppS, \
         tc.tile_pool(name="psO", bufs=2, space="PSUM") as ppO:

        # ---- load constants ----
        xt_s = cpool.tile([128, KO, NP], BF, tag="xt")
        nc.sync.dma_start(xt_s[:], xt.ap())
        wq_s = cpool.tile([128, KO, GH * HD], BF, tag="wq")
        nc.sync.dma_start(wq_s[:], wq.ap())
        wk_s = cpool.tile([128, KO, GH * HD], BF, tag="wk")
        nc.sync.dma_start(wk_s[:], wk.ap())
        wv_s = cpool.tile([128, KO, LH * VW], BF, tag="wv")
        nc.sync.dma_start(wv_s[:], wv.ap())
        pw_s = cpool.tile([64, LH, C], BF, tag="pw")
        nc.sync.dma_start(pw_s[:], pw.ap())
        pb_s = cpool.tile([128, C], F32, tag="pb")
        nc.sync.dma_start(pb_s[:], pb.ap())
        mt_s = cpool.tile([128, ML * NJ, NP], BF, tag="mt")
        nc.sync.dma_start(mt_s[:], mt.ap())
        cw_s = cpool.tile([128, GH * ML * LH], F32, tag="cw")
        nc.sync.dma_start(cw_s[:], cw.ap())
        eb_s = cpool.tile([128, 1], F32, tag="eb")
        nc.sync.dma_start(eb_s[:], eb.ap())

        # ---- phase A: projections ----
        # qT/kT: [d, token] per head; heads g0,g1 packed on partitions 0:64 /
        # 64:128 of one [128, NP] tile, g2 on its own [64, NP] tile.
        qT01 = cpool.tile([128, NP], BF, tag="qT01")
        qT2 = cpool.tile([64, NP], BF, tag="qT2")
        kT01 = cpool.tile([128, NP], BF, tag="kT01")
        kT2 = cpool.tile([64, NP], BF, tag="kT2")
        vt = cpool.tile([128, NJ, LH * VW], BF, tag="vt")

        def proj_to(dst, w_s, mslc, mpart):
            # dst[token-cols] = (W[:, mslc]^T @ xhatT), written as fp16
            for n0, n1 in ((0, 512), (512, NP)):
                ps = ppA.tile([128, 512], F32, tag="bigA", name="psA")[:mpart, : n1 - n0]
                for o in range(KO):
                    nc.tensor.matmul(
                        ps, w_s[:, o, mslc], xt_s[:, o, n0:n1],
                        start=(o == 0), stop=(o == KO - 1),
                    )
                nc.scalar.copy(dst[:mpart, n0:n1], ps)

        proj_to(qT01, wq_s, slice(0, 128), 128)
        proj_to(qT2, wq_s, slice(128, 192), 64)
        proj_to(kT01, wk_s, slice(0, 128), 128)
        proj_to(kT2, wk_s, slice(128, 192), 64)

        # v-hat: rows k (5 chunks of 128), cols = 12 heads x [v | ones]
        for kc in range(NJ):
            for n0, n1 in ((0, 512), (512, LH * VW)):
                ps = ppA.tile([128, 512], F32, tag="bigA", name="psA")[:, : n1 - n0]
                for o in range(KO):
                    nc.tensor.matmul(
                        ps, xt_s[:, o, kc * 128:(kc + 1) * 128], wv_s[:, o, n0:n1],
                        start=(o == 0), stop=(o == KO - 1),
                    )
                nc.scalar.copy(vt[:, kc, n0:n1], ps)

        def qg(g):
            return (qT01[0:64], qT01[64:128], qT2[0:64])[g]

        def kg(g):
            return (kT01[0:64], kT01[64:128], kT2[0:64])[g]

        # ---- phase B: per q-chunk pipeline ----
        for qc in range(NQC):
            qs = slice(qc * QC, (qc + 1) * QC)

            # scores S^T[k, q] for each global head -> bf16 SBUF
            ssb = wpool.tile([128, GH, NJ, QC], BF, tag="ssb")
            for g in range(GH):
                for j in range(NJ):
                    ps = ppS.tile([128, QC], F32, tag="s", name="psS")
                    nc.tensor.matmul(
                        ps, kg(g)[:, j * 128:(j + 1) * 128], qg(g)[:, qs],
                        start=True, stop=True,
                    )
                    nc.scalar.copy(ssb[:, g, j], ps)

            # channel maps c[g,m] = S_g * masks_m
            cmaps = wpool.tile([128, GH * ML, NJ, QC], BF, tag="cmaps")
            for g in range(GH):
                for m in range(ML):
                    nc.vector.tensor_mul(
                        cmaps[:, g * ML + m], ssb[:, g],
                        mt_s[:, m * NJ:(m + 1) * NJ, qs],
                    )

            # per-head mix + exp + p@v
            osb = wpool.tile([64, LH, QC], F32, tag="osb")
            zsb = wpool.tile([65, LH, QC], F32, tag="zsb")
            zrec = wpool.tile([65, LH, QC], F32, tag="zrec")
            for h in range(LH):
                at = apool.tile([128, NJ, QC], BF, tag="at")
                # attn_h = sum_{g,m} mask_proj[m, g*LH+h] * c[g,m]
                i0 = 0  # channel (g=0, m=0)
                nc.vector.tensor_scalar(
                    at[:], cmaps[:, i0], cw_s[:, h:h + 1], None, OP.mult,
                )
                for g in range(GH):
                    for m in range(ML):
                        if g == 0 and m == 0:
                            continue
                        idx = (g * ML + m) * LH + h
                        nc.vector.scalar_tensor_tensor(
                            out=at[:], in0=cmaps[:, g * ML + m],
                            scalar=cw_s[:, idx:idx + 1], in1=at[:],
                            op0=OP.mult, op1=OP.add,
                        )
                e = apool.tile([128, NJ, QC], BF, tag="e")
                nc.scalar.activation(e[:, 0:4], at[:, 0:4], AF.Exp)
                nc.scalar.activation(e[:, 4:5], at[:, 4:5], AF.Exp, bias=eb_s[:, 0:1])

                # o^T_h (and Z in row 64) = vhat_h^T @ e
                pov = ppO.tile([VW, QC], F32, tag="ov")
                for j in range(NJ):
                    nc.tensor.matmul(
                        pov, vt[:, j, h * VW:(h + 1) * VW], e[:, j, :],
                        start=(j == 0), stop=(j == NJ - 1),
                    )
                nc.scalar.copy(osb[:, h], pov[0:64])
                nc.scalar.copy(zsb[64:65, h], pov[64:65])

            # 1/Z, broadcast over the 64 head-dim partitions via DMA
            nc.vector.reciprocal(zrec[64:65], zsb[64:65])
            zrep = wpool.tile([64, LH, QC], F32, tag="zrep")
            nc.sync.dma_start(
                zrep[:], zrec[64:65, None, :, :].to_broadcast((1, 64, LH, QC))
            )
            on = wpool.tile([64, LH, QC], BF, tag="on")
            nc.vector.tensor_mul(on[:], osb[:], zrep[:])

            # final projection for this q-chunk (+ bias)
            outsb = wpool.tile([128, C], F32, tag="outsb")
            for n0, n1 in ((0, 512), (512, C)):
                ps = ppA.tile([128, 512], F32, tag="bigA", name="psA")[:, : n1 - n0]
                for kk in range(LH):
                    nc.tensor.matmul(
                        ps, on[:, kk, :], pw_s[:, kk, n0:n1],
                        start=(kk == 0), stop=(kk == LH - 1),
                    )
                nc.vector.tensor_add(outsb[:, n0:n1], ps, pb_s[:, n0:n1])
            nc.sync.dma_start(
                out.ap().rearrange("(j p) c -> p j c", p=128)[:, qc, :], outsb[:]
            )

    nc.compile()
    return nc


def prep_xt(x_b):
    bf = np.float16
    xhatT = np.zeros((CP, NP), np.float32)
    xhatT[:C, :N] = x_b.T
    xhatT[C, :N] = 1.0
    xt = np.ascontiguousarray(xhatT.reshape(KO, 128, NP).transpose(1, 0, 2))
    return xt.astype(bf)


def prep_shared_inputs(masks, Wq, Wk, Wv, mask_proj, proj_w, proj_b):
    """Host-side prep of the batch-independent input tensors."""
    bf = np.float16

    def wpad(w, scale=1.0):
        wp = np.zeros((CP, w.shape[1]), np.float32)
        wp[:C] = w * scale
        return np.ascontiguousarray(wp.reshape(KO, 128, -1).transpose(1, 0, 2))

    wqp = wpad(Wq, SCALE)
    wkp = wpad(Wk)

    wvh = np.zeros((CP, LH * VW), np.float32)
    for h in range(LH):
        wvh[:C, h * VW:h * VW + HD] = Wv[:, h * HD:(h + 1) * HD]
        wvh[C, h * VW + HD] = 1.0
    wvp = np.ascontiguousarray(wvh.reshape(KO, 128, -1).transpose(1, 0, 2))

    pwp = np.ascontiguousarray(proj_w.reshape(LH, 64, C).transpose(1, 0, 2))
    pbp = np.broadcast_to(proj_b.astype(np.float32), (128, C)).copy()

    # mt[p, m*NJ+j, t] = masks[t, j*128+p, m]  (zero padded)
    mtp = np.zeros((128, ML * NJ, NP), np.float32)
    mk = masks.transpose(2, 1, 0)  # [m, k, q]
    mkp = np.zeros((ML, NP, NP), np.float32)
    mkp[:, :N, :N] = mk
    mtp[:] = mkp.reshape(ML, NJ, 128, NP).transpose(2, 0, 1, 3).reshape(128, ML * NJ, NP)

    # cw[p, (g*ML+m)*LH + h] = mask_proj[m, g*LH + h]
    cwv = np.zeros(GH * ML * LH, np.float32)
    for g in range(GH):
        for m in range(ML):
            for h in range(LH):
                cwv[(g * ML + m) * LH + h] = mask_proj[m, g * LH + h]
    cwp = np.broadcast_to(cwv, (128, GH * ML * LH)).copy()

    ebp = np.zeros((128, 1), np.float32)
    ebp[65:, 0] = EXP_NEG  # k = 512 + p valid through p = 64 (k = 576)

    return {
        "wq": wqp.astype(bf), "wk": wkp.astype(bf),
        "wv": wvp.astype(bf), "pw": pwp.astype(bf), "pb": pbp,
        "mt": mtp.astype(bf), "cw": cwp, "eb": ebp,
    }


_NC = None
_LINEARIZE = False


def get_nc():
    global _NC
    if _NC is None:
        _NC = build_nc(linearize=_LINEARIZE)
    return _NC


def kernel_v1(x, masks, Wq, Wk, Wv, mask_proj, proj_w, proj_b):
    x = np.asarray(x, np.float32)
    masks = np.asarray(masks, np.float32)
    Wq = np.asarray(Wq, np.float32)
    Wk = np.asarray(Wk, np.float32)
    Wv = np.asarray(Wv, np.float32)
    mask_proj = np.asarray(mask_proj, np.float32)
    proj_w = np.asarray(proj_w, np.float32)
    proj_b = np.asarray(proj_b, np.float32)

    shared = prep_shared_inputs(masks, Wq, Wk, Wv, mask_proj, proj_w, proj_b)
    in_maps = [dict(shared, xt=prep_xt(x[b])) for b in range(B)]

    res = bass_utils.run_bass_kernel_spmd(get_nc(), in_maps, core_ids=list(range(B)))
    out = np.stack([r["o"][:N, :] for r in res.results])
    return out.astype(np.float32)




# ======================================================================
# Stage-2 sharding (default): 4 head-groups x 2 query-halves.
# ======================================================================

B, N, C = 8, 577, 768
GH, LH, ML, HD = 3, 12, 3, 64
NH = 3            # heads per core
SCALE = HD ** -0.5
NP, NJ = 640, 5
CP, KO = 896, 7
QW = 320          # q-half width
VW = HD + 1
EXP_NEG = -30.0
QCHUNKS = ((0, 128), (128, 256), (256, 320))  # local q chunks for proj/psum


def build_nc2():
    nc = bacc_mod.Bacc("TRN2", target_bir_lowering=False, debug=False, num_devices=8)

    xta = nc.dram_tensor("xta", [B, 128, KO, NP], BF, kind="ExternalInput")
    xqa = nc.dram_tensor("xqa", [B, 128, KO, QW], BF, kind="ExternalInput")
    wq = nc.dram_tensor("wq", [128, KO, GH * HD], BF, kind="ExternalInput")
    wk = nc.dram_tensor("wk", [128, KO, GH * HD], BF, kind="ExternalInput")
    wv = nc.dram_tensor("wv", [128, KO, NH * VW], BF, kind="ExternalInput")
    pw = nc.dram_tensor("pw", [64, NH, C], BF, kind="ExternalInput")
    mt = nc.dram_tensor("mt", [128, ML * NJ, QW], BF, kind="ExternalInput")
    cw = nc.dram_tensor("cw", [128, GH * ML * NH], F32, kind="ExternalInput")
    eb = nc.dram_tensor("eb", [128, 1], F32, kind="ExternalInput")
    out = nc.dram_tensor("op", [B, QW, C], BF, kind="ExternalOutput")

    with tile.TileContext(nc) as tc, \
         tc.tile_pool(name="const", bufs=1) as cpool, \
         tc.tile_pool(name="xb", bufs=2) as xpool, \
         tc.tile_pool(name="work", bufs=2) as wpool, \
         tc.tile_pool(name="attn", bufs=3) as apool, \
         tc.tile_pool(name="psA", bufs=2, space="PSUM") as ppA, \
         tc.tile_pool(name="psS", bufs=1, space="PSUM") as ppS, \
         tc.tile_pool(name="psO", bufs=1, space="PSUM") as ppO, \
         tc.tile_pool(name="psP", bufs=1, space="PSUM") as ppP:

        wq_s = cpool.tile([128, KO, GH * HD], BF, tag="wq")
        nc.sync.dma_start(wq_s[:], wq.ap())
        wk_s = cpool.tile([128, KO, GH * HD], BF, tag="wk")
        nc.sync.dma_start(wk_s[:], wk.ap())
        wv_s = cpool.tile([128, KO, NH * VW], BF, tag="wv")
        nc.sync.dma_start(wv_s[:], wv.ap())
        pw_s = cpool.tile([64, NH, C], BF, tag="pw")
        nc.sync.dma_start(pw_s[:], pw.ap())
        mt_s = cpool.tile([128, ML * NJ, QW], BF, tag="mt")
        nc.sync.dma_start(mt_s[:], mt.ap())
        cw_s = cpool.tile([128, GH * ML * NH], F32, tag="cw")
        nc.sync.dma_start(cw_s[:], cw.ap())
        eb_s = cpool.tile([128, 1], F32, tag="eb")
        nc.sync.dma_start(eb_s[:], eb.ap())

        # mw[g,hh] = sum_m mask_proj[m, g*LH + H0+hh] * masks_m  (batch-free)
        mwsb = cpool.tile([128, GH * NH, NJ, QW], BF, tag="mw")
        for g in range(GH):
            for hh in range(NH):
                d = mwsb[:, g * NH + hh]
                i0 = (g * ML + 0) * NH + hh
                nc.vector.tensor_scalar(
                    d, mt_s[:, 0:NJ], cw_s[:, i0:i0 + 1], None, OP.mult,
                )
                for m in (1, 2):
                    im = (g * ML + m) * NH + hh
                    nc.vector.scalar_tensor_tensor(
                        out=d, in0=mt_s[:, m * NJ:(m + 1) * NJ],
                        scalar=cw_s[:, im:im + 1], in1=d,
                        op0=OP.mult, op1=OP.add,
                    )

        def phase_a(b):
            xb = xpool.tile([128, KO, NP], BF, tag="xb")
            nc.sync.dma_start(xb[:], xta.ap()[b])
            xq = xpool.tile([128, KO, QW], BF, tag="xq")
            nc.sync.dma_start(xq[:], xqa.ap()[b])

            q01 = wpool.tile([128, QW], BF, tag="q01")
            q2 = wpool.tile([64, QW], BF, tag="q2")
            k01 = wpool.tile([128, NP], BF, tag="k01")
            k2 = wpool.tile([64, NP], BF, tag="k2")
            vtb = wpool.tile([128, NJ, NH * VW], BF, tag="vtb")

            for msl, mp, dst in ((slice(0, 128), 128, q01), (slice(128, 192), 64, q2)):
                ps = ppA.tile([128, 512], F32, tag="bigA", name="psA")[:mp, :QW]
                for o in range(KO):
                    nc.tensor.matmul(ps, wq_s[:, o, msl], xq[:, o, :],
                                     start=(o == 0), stop=(o == KO - 1))
                nc.scalar.copy(dst[:mp, :], ps)

            for msl, mp, dst in ((slice(0, 128), 128, k01), (slice(128, 192), 64, k2)):
                for n0, n1 in ((0, 512), (512, NP)):
                    ps = ppA.tile([128, 512], F32, tag="bigA", name="psA")[:mp, : n1 - n0]
                    for o in range(KO):
                        nc.tensor.matmul(ps, wk_s[:, o, msl], xb[:, o, n0:n1],
                                         start=(o == 0), stop=(o == KO - 1))
                    nc.scalar.copy(dst[:mp, n0:n1], ps)

            for kc in range(NJ):
                ps = ppA.tile([128, 512], F32, tag="bigA", name="psA")[:, : NH * VW]
                for o in range(KO):
                    nc.tensor.matmul(ps, xb[:, o, kc * 128:(kc + 1) * 128], wv_s[:, o, :],
                                     start=(o == 0), stop=(o == KO - 1))
                nc.scalar.copy(vtb[:, kc, :], ps)
            return q01, q2, k01, k2, vtb

        def phase_b(b, q01, q2, k01, k2, vtb):
            def qg(g):
                return (q01[0:64], q01[64:128], q2[0:64])[g]

            def kg(g):
                return (k01[0:64], k01[64:128], k2[0:64])[g]

            ssb = wpool.tile([128, GH, NJ, QW], BF, tag="ssb")
            for g in range(GH):
                psa = ppS.tile([128, NJ, 256], F32, tag="s256", name="psS1")
                psb = ppS.tile([128, NJ, 64], F32, tag="s64", name="psS2")
                for j in range(NJ):
                    nc.tensor.matmul(psa[:, j, :], kg(g)[:, j * 128:(j + 1) * 128],
                                     qg(g)[:, 0:256], start=True, stop=True)
                    nc.tensor.matmul(psb[:, j, :], kg(g)[:, j * 128:(j + 1) * 128],
                                     qg(g)[:, 256:QW], start=True, stop=True)
                nc.scalar.copy(ssb[:, g, :, 0:256], psa)
                nc.scalar.copy(ssb[:, g, :, 256:QW], psb)

            osb = wpool.tile([64, NH, QW], F32, tag="osb")
            zsb = wpool.tile([65, NH, QW], F32, tag="zsb")
            zrec = wpool.tile([65, NH, QW], F32, tag="zrec")
            for hh in range(NH):
                at = apool.tile([128, NJ, QW], BF, tag="at")
                tt = apool.tile([128, NJ, QW], BF, tag="tt")
                nc.vector.tensor_mul(at[:], ssb[:, 0], mwsb[:, 0 * NH + hh])
                for g in (1, 2):
                    nc.vector.tensor_mul(tt[:], ssb[:, g], mwsb[:, g * NH + hh])
                    nc.vector.tensor_add(at[:], at[:], tt[:])
                e = apool.tile([128, NJ, QW], BF, tag="e")
                nc.scalar.activation(e[:, 0:4], at[:, 0:4], AF.Exp)
                nc.scalar.activation(e[:, 4:5], at[:, 4:5], AF.Exp, bias=eb_s[:, 0:1])

                pov = ppO.tile([VW, QW], F32, tag="ov", name="psO1")
                for j in range(NJ):
                    nc.tensor.matmul(pov, vtb[:, j, hh * VW:(hh + 1) * VW], e[:, j, :],
                                     start=(j == 0), stop=(j == NJ - 1))
                nc.scalar.copy(osb[:, hh], pov[0:64])
                nc.scalar.copy(zsb[64:65, hh], pov[64:65])

            nc.vector.reciprocal(zrec[64:65], zsb[64:65])
            zrep = wpool.tile([64, NH, QW], F32, tag="zrep")
            nc.sync.dma_start(
                zrep[:], zrec[64:65, None, :, :].to_broadcast((1, 64, NH, QW)))
            on = wpool.tile([64, NH, QW], BF, tag="on")
            nc.vector.tensor_mul(on[:], osb[:], zrep[:])

            for q0, q1 in QCHUNKS:
                outsb = wpool.tile([128, C], BF, tag="outsb")
                for n0 in range(0, C, 256):
                    ps = ppP.tile([128, 256], F32, tag="prj", name="psP")[: q1 - q0, :]
                    for hh in range(NH):
                        nc.tensor.matmul(ps, on[:, hh, q0:q1], pw_s[:, hh, n0:n0 + 256],
                                         start=(hh == 0), stop=(hh == NH - 1))
                    nc.scalar.copy(outsb[: q1 - q0, n0:n0 + 256], ps)
                nc.sync.dma_start(out.ap()[b, q0:q1, :], outsb[: q1 - q0, :])

        # software pipeline: emit batch b+1's projections before batch b's
        # attention so the PE never stalls behind the ACT/DVE backlog
        prev = phase_a(0)
        for b in range(B):
            nxt = phase_a(b + 1) if b + 1 < B else None
            phase_b(b, *prev)
            prev = nxt

    nc.compile()
    return nc


def prep_inputs2(x, masks, Wq, Wk, Wv, mask_proj, proj_w, proj_b):
    """Returns (in_maps list for 8 cores, host-side finisher info)."""
    f16 = np.float16

    xhatT = np.zeros((B, CP, NP), np.float32)
    xhatT[:, :C, :N] = x.transpose(0, 2, 1)
    xhatT[:, C, :N] = 1.0
    xta = np.ascontiguousarray(
        xhatT.reshape(B, KO, 128, NP).transpose(0, 2, 1, 3)).astype(f16)

    def wpad(w, scale=1.0):
        wp = np.zeros((CP, w.shape[1]), np.float32)
        wp[:C] = w * scale
        return np.ascontiguousarray(wp.reshape(KO, 128, -1).transpose(1, 0, 2)).astype(f16)

    wqp = wpad(Wq, SCALE)
    wkp = wpad(Wk)

    mk = masks.transpose(2, 1, 0)  # [m, k, q]
    mkp = np.zeros((ML, NP, NP), np.float32)
    mkp[:, :N, :N] = mk
    mt_full = mkp.reshape(ML, NJ, 128, NP).transpose(2, 0, 1, 3).reshape(
        128, ML * NJ, NP).astype(f16)

    ebp = np.zeros((128, 1), np.float32)
    ebp[65:, 0] = EXP_NEG

    in_maps = []
    for c in range(8):
        hg, s = c // 2, c % 2
        H0 = NH * hg
        qo = QW * s

        wvh = np.zeros((CP, NH * VW), np.float32)
        for hh in range(NH):
            h = H0 + hh
            wvh[:C, hh * VW:hh * VW + HD] = Wv[:, h * HD:(h + 1) * HD]
            wvh[C, hh * VW + HD] = 1.0
        wvp = np.ascontiguousarray(
            wvh.reshape(KO, 128, -1).transpose(1, 0, 2)).astype(f16)

        pwp = np.ascontiguousarray(
            proj_w.reshape(LH, 64, C)[H0:H0 + NH].transpose(1, 0, 2)).astype(f16)

        cwv = np.zeros(GH * ML * NH, np.float32)
        for g in range(GH):
            for m in range(ML):
                for hh in range(NH):
                    cwv[(g * ML + m) * NH + hh] = mask_proj[m, g * LH + H0 + hh]
        cwp = np.broadcast_to(cwv, (128, GH * ML * NH)).copy()

        in_maps.append({
            "xta": xta,
            "xqa": np.ascontiguousarray(xta[:, :, :, qo:qo + QW]),
            "wq": wqp, "wk": wkp, "wv": wvp, "pw": pwp,
            "mt": np.ascontiguousarray(mt_full[:, :, qo:qo + QW]),
            "cw": cwp, "eb": ebp,
        })
    return in_maps


_NC2 = None


def get_nc2():
    global _NC2
    if _NC2 is None:
        _NC2 = build_nc2()
    return _NC2


def kernel_v2(x, masks, Wq, Wk, Wv, mask_proj, proj_w, proj_b):
    x = np.asarray(x, np.float32)
    in_maps = prep_inputs2(
        x, np.asarray(masks, np.float32), np.asarray(Wq, np.float32),
        np.asarray(Wk, np.float32), np.asarray(Wv, np.float32),
        np.asarray(mask_proj, np.float32), np.asarray(proj_w, np.float32),
        np.asarray(proj_b, np.float32))
    res = bass_utils.run_bass_kernel_spmd(get_nc2(), in_maps, core_ids=list(range(8)))
    # sum the 4 head-group partials per q-half, concat halves, add bias
    out = np.zeros((B, NP, C), np.float32)
    for c in range(8):
        hg, s = c // 2, c % 2
        out[:, QW * s:QW * (s + 1), :] += np.asarray(
            res.results[c]["op"], np.float32)
    out = out[:, :N, :] + np.asarray(proj_b, np.float32)
    return out.astype(np.float32)

def kernel(x, masks, Wq, Wk, Wv, mask_proj, proj_w, proj_b):
    return kernel_v2(x, masks, Wq, Wk, Wv, mask_proj, proj_w, proj_b)


if __name__ == "__main__":
    rng = np.random.default_rng(0)
    ins = {
        "x": rng.standard_normal((B, N, C)).astype(np.float32),
        "masks": rng.random((N, N, ML)).astype(np.float32),
        "Wq": (rng.standard_normal((C, GH * HD)) * 0.02).astype(np.float32),
        "Wk": (rng.standard_normal((C, GH * HD)) * 0.02).astype(np.float32),
        "Wv": (rng.standard_normal((C, C)) * 0.02).astype(np.float32),
        "mask_proj": (rng.standard_normal((ML, GH * LH)) * 0.5 + 1.0).astype(np.float32),
        "proj_w": (rng.standard_normal((C, C)) * 0.02).astype(np.float32),
        "proj_b": (rng.standard_normal(C) * 0.02).astype(np.float32),
    }
    out = kernel(**ins)
    print(out.shape, out.dtype)



# revision 2
# speedup vs baseline: 1.6589x; 1.6589x over previous
# Delegates to kernel3 for local testing; will be inlined before delivery.
from kernel3 import *  # noqa: F401,F403
from kernel3 import kernel3


def kernel(x, masks, Wq, Wk, Wv, mask_proj, proj_w, proj_b):
    return kernel3(x, masks, Wq, Wk, Wv, mask_proj, proj_w, proj_b)
